# revision 6
# baseline (speedup 1.0000x reference)
"""Trainium2 Bass kernel for nn_CapsuleLayer (dynamic routing).

Math (per independent column c=(k,b,o), vector p = pred[k,b,:,o] of length N):
    logits stay proportional to p:  logits_t = p * V_t  with scalar V_t.
    iter 1: c uniform -> s1 = mean_n(p);  v1 = squash(s1); V1 = v1
    iter t: Z = sum_n exp(V*p), Y = sum_n p*exp(V*p), s = Y/Z,
            v = squash(s) = s*|s|/(1+s^2), V += v
    output = v from the last iteration.

Sharding: data-parallel over batch (32 of 256 per core, 8 cores).

Per-core device pipeline:
  pred is computed by PE as  Wr[ng].T @ xbd[bg,ng]  where xbd is a
  host-built block-diagonal slab of x (contraction = 16 n-values x 8 cin),
  so PSUM comes out column-major: rows=(k,o), free=(nl, bl).
    - "T1" rows = ko 0..127 (k0..7)  -> A1[bg] tiles [128, 8*1152] fp16
    - "A2" rows = ko 128..159 (k8,k9), j-packed as (j=ng%4, ko2) with
      free (bg, bl, nq, nl)          -> A2 tile [128, 9216] fp16
  Routing: ScalarE exp (per-partition scale=V, fused accum_out=Z) +
  VectorE tensor_tensor_reduce (Y = sum p*e).  A2 j-partials are fixed up
  with tiny selector matmuls (j-reduce of Z/Y, j-broadcast of V).
"""

import sys

sys.path.insert(0, "/opt/trn_rl_repo")

from contextlib import ExitStack

import numpy as np

import concourse.bass as bass  # noqa: F401
import concourse.bacc as bacc
import concourse.tile as tile
from concourse import mybir
from concourse.bass_utils import run_bass_kernel_spmd

# ---- problem constants (hardcoded per harness contract) ----
B, N, CIN = 256, 1152, 8
K, O = 10, 16
KO = K * O            # 160
NCORES = 8
BSH = B // NCORES     # 32 batch per core
BG, BL = 4, 8         # batch groups x lanes (BSH = BG*BL)
NG, NL = 72, 16       # n-groups x n-lanes (N = NG*NL)
NQ = NG // 4          # 18
F32 = mybir.dt.float32
F16 = mybir.dt.float16

_cache = {}


# ----------------------------------------------------------------------------
# host-side input prep
# ----------------------------------------------------------------------------
def _prep_shared(w):
    # Wr[ng, 8*nl+i, 16*k+o] = w[k, 16*ng+nl, i, o]
    wr = np.transpose(
        w.reshape(K, NG, NL, CIN, O), (1, 2, 3, 0, 4)
    ).reshape(NG, 128, KO).astype(np.float16)
    ident32 = np.eye(32, dtype=np.float32)
    # psB rows = (j, ko2): sel1[(j,ko2), ko2'] = (ko2==ko2')  -> j-reduce
    sel1 = np.tile(np.eye(32, dtype=np.float32), (4, 1))      # [128, 32]
    # sel2[ko2, (j,ko2')] = (ko2==ko2')                        -> j-bcast
    sel2 = np.tile(np.eye(32, dtype=np.float32), (1, 4))      # [32, 128]
    return wr, ident32, sel1, sel2


def _prep_core_inputs(x, w):
    wr, ident32, sel1, sel2 = _prep_shared(w)
    in_maps = []
    for c in range(NCORES):
        xc = x[c * BSH:(c + 1) * BSH]                          # [32, N, CIN]
        # xs[ng, 8*nl+i, b] = xc[b, 16*ng+nl, i]
        xs = np.transpose(
            xc.reshape(BSH, NG, NL, CIN), (1, 2, 3, 0)
        ).reshape(NG, 128, BSH).astype(np.float16)
        # xbd[bg, ng, (nl',i), (nl,bl)] = xc[8bg+bl, 16ng+nl, i] * (nl==nl')
        xbd = np.zeros((BG, NG, NL, CIN, NL, BL), dtype=np.float16)
        xs5 = np.transpose(
            xc.reshape(BG, BL, NG, NL, CIN), (0, 2, 3, 4, 1)
        ).astype(np.float16)                                   # [bg,ng,nl,i,bl]
        for r in range(NL):
            xbd[:, :, r, :, r, :] = xs5[:, :, r, :, :]
        xbd = xbd.reshape(BG, NG, 128, 128)
        in_maps.append({
            "xbd": xbd, "xs": xs, "wr": wr,
            "sel1": sel1, "sel2": sel2, "ident32": ident32,
        })
    return in_maps


# ----------------------------------------------------------------------------
# device program
# ----------------------------------------------------------------------------
def _interleave(*gens):
    gens = list(gens)
    while gens:
        nxt = []
        for g in gens:
            try:
                next(g)
                nxt.append(g)
            except StopIteration:
                pass
        gens = nxt


def _build_program(T):
    nc = bacc.Bacc("TRN2", target_bir_lowering=False, debug=False,
                   enable_asserts=False)

    xbd_d = nc.dram_tensor("xbd", [BG, NG, 128, 128], F16, kind="ExternalInput").ap()
    xs_d = nc.dram_tensor("xs", [NG, 128, BSH], F16, kind="ExternalInput").ap()
    wr_d = nc.dram_tensor("wr", [NG, 128, KO], F16, kind="ExternalInput").ap()
    sel1_d = nc.dram_tensor("sel1", [128, 32], F32, kind="ExternalInput").ap()
    sel2_d = nc.dram_tensor("sel2", [32, 128], F32, kind="ExternalInput").ap()
    id_d = nc.dram_tensor("ident32", [32, 32], F32, kind="ExternalInput").ap()
    out1_d = nc.dram_tensor("out1", [BG, 128, BL], F32, kind="ExternalOutput").ap()
    out2_d = nc.dram_tensor("out2", [32, 32], F32, kind="ExternalOutput").ap()

    mult = mybir.AluOpType.mult
    add = mybir.AluOpType.add
    EXP = mybir.ActivationFunctionType.Exp

    with tile.TileContext(nc) as tc, ExitStack() as ctx:
        consts = ctx.enter_context(tc.tile_pool(name="consts", bufs=1))
        a1p = ctx.enter_context(tc.tile_pool(name="a1", bufs=3))
        a2p = ctx.enter_context(tc.tile_pool(name="a2", bufs=1))
        xbdp = ctx.enter_context(tc.tile_pool(name="xbd", bufs=3))
        ep = ctx.enter_context(tc.tile_pool(name="e", bufs=3))
        scrp = ctx.enter_context(tc.tile_pool(name="scr", bufs=2))
        smp = ctx.enter_context(tc.tile_pool(name="sm", bufs=10))
        psA = ctx.enter_context(tc.tile_pool(name="psA", bufs=2, space="PSUM"))
        psB = ctx.enter_context(tc.tile_pool(name="psB", bufs=2, space="PSUM"))
        psM = ctx.enter_context(tc.tile_pool(name="psM", bufs=1, space="PSUM"))
        psT = ctx.enter_context(tc.tile_pool(name="psT", bufs=2, space="PSUM"))

        # ---- resident inputs ----
        wrs = consts.tile([128, NG * KO], F16, tag="wrs", name="wrs")
        nc.sync.dma_start(
            wrs[:].rearrange("p (g f) -> p g f", g=NG),
            wr_d.rearrange("g p f -> p g f"))
        xsal = consts.tile([128, NG * BSH], F16, tag="xsal", name="xsal")
        nc.sync.dma_start(
            xsal[:].rearrange("p (g f) -> p g f", g=NG),
            xs_d.rearrange("g p f -> p g f"))
        sel1s = consts.tile([128, 32], F32, tag="sel1", name="sel1")
        nc.sync.dma_start(sel1s[:], sel1_d)
        sel2s = consts.tile([32, 128], F32, tag="sel2", name="sel2")
        nc.sync.dma_start(sel2s[:], sel2_d)
        id32 = consts.tile([32, 32], F32, tag="id32", name="id32")
        nc.sync.dma_start(id32[:], id_d)

        # A2 accumulator: rows (j, ko2); free (bg, bl, nq, nl)
        a2t = a2p.tile([128, BG * BL * NQ * NL], F16, tag="a2", name="a2")

        st = {}
        evac_tgl = [0]

        def evac_copy(dst, src):
            # alternate PSUM->SBUF evacuation between ScalarE and VectorE
            if evac_tgl[0] % 3 == 0:
                nc.scalar.copy(dst, src)
            else:
                nc.vector.tensor_copy(dst, src)
            evac_tgl[0] += 1

        def squash(s_ap, P, W):
            """v = s*|s|/(1+s*s) as a fresh [P, W] f32 tile"""
            n2 = smp.tile([P, W], F32, tag=f"sq_n2_{P}_{W}", name=f"sq_n2_{P}_{W}")
            nc.vector.tensor_tensor(n2[:], s_ap, s_ap, mult)
            d = smp.tile([P, W], F32, tag=f"sq_d_{P}_{W}", name=f"sq_d_{P}_{W}")
            nc.vector.tensor_scalar_add(d[:], n2[:], 1.0)
            r = smp.tile([P, W], F32, tag=f"sq_r_{P}_{W}", name=f"sq_r_{P}_{W}")
            nc.vector.reciprocal(r[:], d[:])
            a = smp.tile([P, W], F32, tag=f"sq_a_{P}_{W}", name=f"sq_a_{P}_{W}")
            nc.scalar.activation(a[:], s_ap, mybir.ActivationFunctionType.Abs)
            t = smp.tile([P, W], F32, tag=f"sq_t_{P}_{W}", name=f"sq_t_{P}_{W}")
            nc.vector.tensor_tensor(t[:], s_ap, a[:], mult)
            v = smp.tile([P, W], F32, tag=f"sq_v_{P}_{W}", name=f"sq_v_{P}_{W}")
            nc.vector.tensor_tensor(v[:], t[:], r[:], mult)
            return v

        # ------------------------------------------------------------------
        def gen_phase(bg):
            a1t = a1p.tile([128, BL * N], F16, tag="a1", name="a1")
            st[("a1", bg)] = a1t
            a1v = a1t[:].rearrange("p (b g l) -> p g l b", b=BL, g=NG, l=NL)
            a2v = a2t[:].rearrange("p (G b q l) -> p q l b G",
                                   G=BG, b=BL, q=NQ, l=NL)
            pb = None
            pb_q0 = 0
            for Qn in range(NQ):              # 18 blocks of 4 ng
                xb = xbdp.tile([128, 4 * 128], F16, tag="xbd", name="xbd")
                nc.sync.dma_start(
                    xb[:].rearrange("p (n f) -> p n f", n=4),
                    xbd_d[bg, 4 * Qn:4 * Qn + 4].rearrange("n p f -> p n f"))
                pa = psA.tile([128, 512], F32, tag="psA", name="psA")
                if Qn % 4 == 0:
                    pb = psB.tile([128, 512], F32, tag="psB", name="psB")
                    pb_q0 = Qn
                for j in range(4):
                    ng = 4 * Qn + j
                    w0 = wrs[:, ng * KO:ng * KO + 128]
                    w1 = wrs[:, ng * KO + 128:ng * KO + KO]
                    rhs = xb[:, j * 128:(j + 1) * 128]
                    nc.tensor.matmul(pa[:, j * 128:(j + 1) * 128],
                                     w0, rhs, start=True, stop=True)
                    q = Qn - pb_q0
                    nc.tensor.matmul(
                        pb[32 * j:32 * j + 32, q * 128:(q + 1) * 128],
                        w1, rhs, start=True, stop=True,
                        tile_position=(0, 32 * j))
                    if bg == 0:
                        nc.tensor.matmul(
                            st["m1ps"][:],
                            xsal[:, ng * BSH:(ng + 1) * BSH],
                            wrs[:, ng * KO:(ng + 1) * KO],
                            start=(ng == 0), stop=(ng == NG - 1))
                # evacuate psA -> A1[bg]; dst/src iteration order = (g, l, b)
                evac_copy(a1v[:, 4 * Qn:4 * Qn + 4, :, :],
                          pa[:].rearrange("p (g l b) -> p g l b",
                                          g=4, l=NL, b=BL))
                if Qn % 4 == 3 or Qn == NQ - 1:
                    ncnt = Qn - pb_q0 + 1
                    evac_copy(
                        a2v[:, pb_q0:pb_q0 + ncnt, :, :, bg],
                        pb[:, :ncnt * 128].rearrange(
                            "p (q l b) -> p q l b", q=ncnt, l=NL, b=BL))
                yield

        # ------------------------------------------------------------------
        def m1_finalize():
            m1s = smp.tile([32, KO], F32, tag="m1s", name="m1s")
            nc.vector.tensor_copy(m1s[:], st["m1ps"][:])
            t1 = psT.tile([128, 32], F32, tag="psT", name="psT")
            nc.tensor.transpose(t1[:], m1s[:, 0:128], id32[:])
            t2 = psT.tile([128, 32], F32, tag="psT", name="psT")
            nc.tensor.transpose(t2[:32, :], m1s[:, 128:KO], id32[:])
            s1 = smp.tile([128, 32], F32, tag="s1t1", name="s1t1")
            nc.vector.tensor_scalar_mul(s1[:], t1[:], 1.0 / N)
            v1 = squash(s1[:], 128, 32)
            st["V_t1_1"] = v1              # [128, (bg,bl)]
            s1a = smp.tile([32, 32], F32, tag="s1a2", name="s1a2")
            nc.vector.tensor_scalar_mul(s1a[:], t2[:32, :], 1.0 / N)
            v1a = squash(s1a[:], 32, 32)
            st["Va2_small_1"] = v1a        # [32(ko2), 32(bg,bl)]
            vb = psT.tile([128, 32], F32, tag="psT", name="psT")
            nc.tensor.matmul(vb[:], sel2s[:], v1a[:], start=True, stop=True)
            vbig = smp.tile([128, 32], F32, tag="va2big", name="va2big")
            nc.vector.tensor_copy(vbig[:], vb[:])
            st["Va2_big_1"] = vbig         # [128(j,ko2), 32(bg,bl)]
            yield

        # ------------------------------------------------------------------
        def route_t1(bg):
            a1t = st[("a1", bg)]
            vl = None
            for it in range(2, T + 1):
                if it == 2:
                    Vfull = st["V_t1_1"]
                    voff = 8 * bg
                else:
                    Vfull = st[("V_t1", bg, it - 1)]
                    voff = 0
                Z = smp.tile([128, BL], F32, tag="Zt1", name="Zt1")
                Y = smp.tile([128, BL], F32, tag="Yt1", name="Yt1")
                for bl in range(BL):
                    p_sl = a1t[:, bl * N:(bl + 1) * N]
                    e = ep.tile([128, N], F16, tag="e", name="e")
                    nc.scalar.activation(
                        e[:], p_sl, EXP,
                        scale=Vfull[:, voff + bl:voff + bl + 1],
                        accum_out=Z[:, bl:bl + 1])
                    scr = scrp.tile([128, N], F16, tag="scr", name="scr")
                    nc.vector.scalar_tensor_tensor(
                        out=scr[:], in0=p_sl, scalar=1.0, in1=e[:],
                        op0=mult, op1=mult, accum_out=Y[:, bl:bl + 1])
                    if bl % 2 == 1:
                        yield
                r = smp.tile([128, BL], F32, tag="rt1", name="rt1")
                nc.vector.reciprocal(r[:], Z[:])
                s = smp.tile([128, BL], F32, tag="st1", name="st1")
                nc.vector.tensor_tensor(s[:], Y[:], r[:], mult)
                v = squash(s[:], 128, BL)
                Vn = smp.tile([128, BL], F32, tag="Vt1n", name="Vt1n")
                nc.vector.tensor_tensor(
                    Vn[:], Vfull[:, voff:voff + BL], v[:], add)
                st[("V_t1", bg, it)] = Vn
                vl = v
                yield
            if T == 1:
                vlast = st["V_t1_1"][:, 8 * bg:8 * bg + 8]
            else:
                vlast = vl[:]
            nc.sync.dma_start(out1_d[bg], vlast)
            yield

        # ------------------------------------------------------------------
        def route_a2():
            vl = None
            for it in range(2, T + 1):
                Vbig = st["Va2_big_%d" % (it - 1)]
                Vsm = st["Va2_small_%d" % (it - 1)]
                Z = smp.tile([128, 32], F32, tag="Za2", name="Za2")
                Y = smp.tile([128, 32], F32, tag="Ya2", name="Ya2")
                for bg in range(BG):
                    for bl in range(BL):
                        col = bg * BL + bl
                        off = col * NQ * NL
                        p_sl = a2t[:, off:off + NQ * NL]
                        e = ep.tile([128, N], F16, tag="e", name="e")
                        nc.scalar.activation(
                            e[:, :NQ * NL], p_sl, EXP,
                            scale=Vbig[:, col:col + 1],
                            accum_out=Z[:, col:col + 1])
                        scr = scrp.tile([128, N], F16, tag="scr", name="scr")
                        nc.vector.scalar_tensor_tensor(
                            out=scr[:, :NQ * NL], in0=p_sl, scalar=1.0,
                            in1=e[:, :NQ * NL],
                            op0=mult, op1=mult, accum_out=Y[:, col:col + 1])
                        if bl % 2 == 1:
                            yield
                zr = psT.tile([128, 32], F32, tag="psT", name="psT")
                nc.tensor.matmul(zr[:32, :], sel1s[:], Z[:],
                                 start=True, stop=True)
                yr = psT.tile([128, 32], F32, tag="psT", name="psT")
                nc.tensor.matmul(yr[:32, :], sel1s[:], Y[:],
                                 start=True, stop=True)
                zs = smp.tile([32, 32], F32, tag="zs", name="zs")
                nc.vector.tensor_copy(zs[:], zr[:32, :])
                ys = smp.tile([32, 32], F32, tag="ys", name="ys")
                nc.vector.tensor_copy(ys[:], yr[:32, :])
                r = smp.tile([32, 32], F32, tag="ra2", name="ra2")
                nc.vector.reciprocal(r[:], zs[:])
                s = smp.tile([32, 32], F32, tag="sa2", name="sa2")
                nc.vector.tensor_tensor(s[:], ys[:], r[:], mult)
                v = squash(s[:], 32, 32)
                Vn = smp.tile([32, 32], F32, tag="Va2n", name="Va2n")
                nc.vector.tensor_tensor(Vn[:], Vsm[:], v[:], add)
                st["Va2_small_%d" % it] = Vn
                vl = v
                if it < T:
                    vb = psT.tile([128, 32], F32, tag="psT", name="psT")
                    nc.tensor.matmul(vb[:], sel2s[:], Vn[:],
                                     start=True, stop=True)
                    vbig = smp.tile([128, 32], F32, tag="va2big", name="va2big")
                    nc.vector.tensor_copy(vbig[:], vb[:])
                    st["Va2_big_%d" % it] = vbig
                yield
            vlast = st["Va2_small_1"][:] if T == 1 else vl[:]
            nc.sync.dma_start(out2_d, vlast)
            yield

        # ---- emission schedule (pipelined: route(bg-1) || gen(bg)) ----
        import os
        phase = os.environ.get("KCAP_PHASE", "full")
        st["m1ps"] = psM.tile([32, KO], F32, tag="m1ps", name="m1ps")
        if phase == "gen":
            _interleave(gen_phase(0))
            for bg in range(1, BG):
                _interleave(gen_phase(bg))
            # dump a slice of A1/A2 so nothing is dead
            dbg = smp.tile([128, 32], F32, tag="dbg", name="dbg")
            nc.vector.tensor_copy(dbg[:], st[("a1", 0)][:, 0:32])
            nc.sync.dma_start(out2_d, dbg[:32, :])
            nc.sync.dma_start(out1_d[0], dbg[:, :8])
        elif phase == "m1":
            _interleave(gen_phase(0))
            _interleave(m1_finalize())
            for bg in range(1, BG):
                _interleave(gen_phase(bg))
            nc.sync.dma_start(out1_d[0], st["V_t1_1"][:, :8])
            nc.sync.dma_start(out2_d, st["Va2_small_1"][:])
        else:
            _interleave(gen_phase(0))
            _interleave(m1_finalize())
            for bg in range(1, BG):
                _interleave(gen_phase(bg), route_t1(bg - 1))
            _interleave(route_t1(BG - 1), route_a2())

    nc.compile()
    return nc


def _get_program(T):
    if T not in _cache:
        _cache[T] = _build_program(T)
    return _cache[T]


# ----------------------------------------------------------------------------
# host-side output assembly
# ----------------------------------------------------------------------------
def _assemble(results):
    v = np.zeros((K, B, 1, 1, O), dtype=np.float32)
    for c, res in enumerate(results):
        o1 = res["out1"]          # [BG, 128=(16k+o), BL]
        o2 = res["out2"]          # [32=(16(k-8)+o), 32=(bg,bl)]
        b0 = c * BSH
        # o1[bg, 16k+o, bl] -> v[k, b0+8bg+bl, 0, 0, o]
        t = o1.reshape(BG, 8, O, BL).transpose(1, 0, 3, 2)  # [k, bg, bl, o]
        v[:8, b0:b0 + BSH, 0, 0, :] = t.reshape(8, BSH, O)
        # o2[16kk+o, 8bg+bl] -> v[8+kk, b0+8bg+bl, 0, 0, o]
        t2 = o2.reshape(2, O, BSH).transpose(0, 2, 1)       # [kk, b, o]
        v[8:, b0:b0 + BSH, 0, 0, :] = t2
    return v


def run(x, routing_weights, num_iterations, trace=False):
    T = int(num_iterations)
    x = np.asarray(x, dtype=np.float32)
    w = np.asarray(routing_weights, dtype=np.float32)
    nc = _get_program(T)
    in_maps = _prep_core_inputs(x, w)
    kw = {}
    if trace:
        kw = dict(trace=True, trace_cores=list(range(NCORES)))
    res = run_bass_kernel_spmd(nc, in_maps, core_ids=list(range(NCORES)), **kw)
    return _assemble(res.results), res


def kernel(x, routing_weights, num_iterations):
    out, _ = run(x, routing_weights, num_iterations)
    return out


# revision 16
# speedup vs baseline: 47.0454x; 47.0454x over previous
"""Trainium2 Bass kernel for nn_CapsuleLayer (dynamic routing).

Math (per independent column c=(k,b,o), vector p = pred[k,b,:,o] of length N):
    logits stay proportional to p:  logits_t = p * V_t  with scalar V_t.
    iter 1: c uniform -> s1 = mean_n(p);  v1 = squash(s1); V1 = v1
    iter t: Z = sum_n exp(V*p), Y = sum_n p*exp(V*p), s = Y/Z,
            v = squash(s) = s*|s|/(1+s^2), V += v
    output = v from the last iteration.

Sharding: data-parallel over batch (32 of 256 per core, 8 cores).

Per-core device pipeline:
  pred is computed by PE as  Wr[ng].T @ xbd[bg,ng]  where xbd is a
  host-built block-diagonal slab of x (contraction = 16 n-values x 8 cin),
  so PSUM comes out column-major: rows=(k,o), free=(nl, bl).
    - "T1" rows = ko 0..127 (k0..7)  -> A1[bg] tiles [128, 8*1152] fp16
    - "A2" rows = ko 128..159 (k8,k9), j-packed as (j=ng%4, ko2) with
      free (bg, bl, nq, nl)          -> A2 tile [128, 9216] fp16
  Routing: ScalarE exp (per-partition scale=V, fused accum_out=Z) +
  VectorE tensor_tensor_reduce (Y = sum p*e).  A2 j-partials are fixed up
  with tiny selector matmuls (j-reduce of Z/Y, j-broadcast of V).
"""

import sys

sys.path.insert(0, "/opt/trn_rl_repo")

from contextlib import ExitStack

import numpy as np

import concourse.bass as bass  # noqa: F401
import concourse.bacc as bacc
import concourse.tile as tile
from concourse import mybir
from concourse.bass_utils import run_bass_kernel_spmd

# ---- problem constants (hardcoded per harness contract) ----
B, N, CIN = 256, 1152, 8
K, O = 10, 16
KO = K * O            # 160
NCORES = 8
BSH = B // NCORES     # 32 batch per core
BG, BL = 4, 8         # batch groups x lanes (BSH = BG*BL)
NG, NL = 72, 16       # n-groups x n-lanes (N = NG*NL)
NQ = NG // 4          # 18
F32 = mybir.dt.float32
F16 = mybir.dt.float16

_cache = {}


# ----------------------------------------------------------------------------
# host-side input prep
# ----------------------------------------------------------------------------
def _prep_shared(w):
    # Wr[ng, 8*nl+i, 16*k+o] = w[k, 16*ng+nl, i, o]; ship partition-major
    wr = np.transpose(
        w.reshape(K, NG, NL, CIN, O), (1, 2, 3, 0, 4)
    ).reshape(NG, 128, KO).astype(np.float16)
    wr = np.ascontiguousarray(np.transpose(wr, (1, 0, 2)).reshape(128, NG * KO))
    ident32 = np.eye(32, dtype=np.float32)
    # psB rows = (j, ko2): sel1[(j,ko2), ko2'] = (ko2==ko2')  -> j-reduce
    sel1 = np.tile(np.eye(32, dtype=np.float32), (4, 1))      # [128, 32]
    # sel2[ko2, (j,ko2')] = (ko2==ko2')                        -> j-bcast
    sel2 = np.tile(np.eye(32, dtype=np.float32), (1, 4))      # [32, 128]
    return wr, ident32, sel1, sel2


def _prep_core_inputs(x, w):
    wr, ident32, sel1, sel2 = _prep_shared(w)
    in_maps = []
    for c in range(NCORES):
        xc = x[c * BSH:(c + 1) * BSH]                          # [32, N, CIN]
        # xs[ng, 8*nl+i, b] = xc[b, 16*ng+nl, i]
        xs = np.transpose(
            xc.reshape(BSH, NG, NL, CIN), (1, 2, 3, 0)
        ).reshape(NG, 128, BSH).astype(np.float16)
        xs = np.ascontiguousarray(
            np.transpose(xs, (1, 0, 2)).reshape(128, NG * BSH))
        # xbd[bg, ng, (nl',i), (nl,bl)] = xc[8bg+bl, 16ng+nl, i] * (nl==nl')
        xbd = np.zeros((BG, NG, NL, CIN, NL, BL), dtype=np.float16)
        xs5 = np.transpose(
            xc.reshape(BG, BL, NG, NL, CIN), (0, 2, 3, 4, 1)
        ).astype(np.float16)                                   # [bg,ng,nl,i,bl]
        for r in range(NL):
            xbd[:, :, r, :, r, :] = xs5[:, :, r, :, :]
        xbd = np.ascontiguousarray(
            np.transpose(xbd.reshape(BG, NG, 128, 128),
                         (0, 2, 1, 3)).reshape(BG, 128, NG * 128))
        in_maps.append({
            "xbd": xbd, "xs": xs, "wr": wr,
            "sel1": sel1, "sel2": sel2, "ident32": ident32,
        })
    return in_maps


# ----------------------------------------------------------------------------
# device program
# ----------------------------------------------------------------------------
def _interleave(*gens):
    gens = list(gens)
    while gens:
        nxt = []
        for g in gens:
            try:
                next(g)
                nxt.append(g)
            except StopIteration:
                pass
        gens = nxt


def _build_program(T):
    nc = bacc.Bacc("TRN2", target_bir_lowering=False, debug=False,
                   enable_asserts=False)

    xbd_d = nc.dram_tensor("xbd", [BG, 128, NG * 128], F16, kind="ExternalInput").ap()
    xs_d = nc.dram_tensor("xs", [128, NG * BSH], F16, kind="ExternalInput").ap()
    wr_d = nc.dram_tensor("wr", [128, NG * KO], F16, kind="ExternalInput").ap()
    sel1_d = nc.dram_tensor("sel1", [128, 32], F32, kind="ExternalInput").ap()
    sel2_d = nc.dram_tensor("sel2", [32, 128], F32, kind="ExternalInput").ap()
    id_d = nc.dram_tensor("ident32", [32, 32], F32, kind="ExternalInput").ap()
    out1_d = nc.dram_tensor("out1", [BG, 128, BL], F32, kind="ExternalOutput").ap()
    out2_d = nc.dram_tensor("out2", [32, 32], F32, kind="ExternalOutput").ap()

    mult = mybir.AluOpType.mult
    add = mybir.AluOpType.add
    EXP = mybir.ActivationFunctionType.Exp

    with tile.TileContext(nc) as tc, ExitStack() as ctx:
        consts = ctx.enter_context(tc.tile_pool(name="consts", bufs=1))
        a1p = ctx.enter_context(tc.tile_pool(name="a1", bufs=3))
        a2p = ctx.enter_context(tc.tile_pool(name="a2", bufs=1))
        xbdp = ctx.enter_context(tc.tile_pool(name="xbd", bufs=2))
        ep = ctx.enter_context(tc.tile_pool(name="e", bufs=4))
        scrp = ctx.enter_context(tc.tile_pool(name="scr", bufs=3))
        smp = ctx.enter_context(tc.tile_pool(name="sm", bufs=10))
        psA = ctx.enter_context(tc.tile_pool(name="psA", bufs=2, space="PSUM"))
        psB = ctx.enter_context(tc.tile_pool(name="psB", bufs=2, space="PSUM"))
        psM = ctx.enter_context(tc.tile_pool(name="psM", bufs=1, space="PSUM"))
        psT = ctx.enter_context(tc.tile_pool(name="psT", bufs=1, space="PSUM"))

        # ---- resident inputs ----
        wrs = consts.tile([128, NG * KO], F16, tag="wrs", name="wrs")
        xsal = consts.tile([128, NG * BSH], F16, tag="xsal", name="xsal")
        WCH = 4
        for ch in range(WCH):
            g0, g1 = ch * NG // WCH, (ch + 1) * NG // WCH
            nc.sync.dma_start(wrs[:, g0 * KO:g1 * KO], wr_d[:, g0 * KO:g1 * KO])
            nc.sync.dma_start(xsal[:, g0 * BSH:g1 * BSH],
                              xs_d[:, g0 * BSH:g1 * BSH])
        sel1s = consts.tile([128, 32], F32, tag="sel1", name="sel1")
        nc.sync.dma_start(sel1s[:], sel1_d)
        sel2s = consts.tile([32, 128], F32, tag="sel2", name="sel2")
        nc.sync.dma_start(sel2s[:], sel2_d)
        id32 = consts.tile([32, 32], F32, tag="id32", name="id32")
        nc.sync.dma_start(id32[:], id_d)
        ones1 = consts.tile([128, 1], F32, tag="ones1", name="ones1")
        nc.vector.memset(ones1[:], 1.0)

        # A2 accumulator: rows (j, ko2); free (bg, bl, nq, nl)
        a2t = a2p.tile([128, BG * BL * NQ * NL], F16, tag="a2", name="a2")

        st = {}
        evac_tgl = [0]

        def evac_copy(dst, src):
            # alternate PSUM->SBUF evacuation between ScalarE and VectorE
            if evac_tgl[0] % 3 == 0:
                nc.scalar.copy(dst, src)
            else:
                nc.vector.tensor_copy(dst, src)
            evac_tgl[0] += 1

        def squash(s_ap, P, W):
            """v = s*|s|/(1+s*s) as a fresh [P, W] f32 tile"""
            n2 = smp.tile([P, W], F32, tag=f"sq_n2_{P}_{W}", name=f"sq_n2_{P}_{W}")
            nc.vector.tensor_tensor(n2[:], s_ap, s_ap, mult)
            d = smp.tile([P, W], F32, tag=f"sq_d_{P}_{W}", name=f"sq_d_{P}_{W}")
            nc.vector.tensor_scalar_add(d[:], n2[:], 1.0)
            r = smp.tile([P, W], F32, tag=f"sq_r_{P}_{W}", name=f"sq_r_{P}_{W}")
            nc.vector.reciprocal(r[:], d[:])
            a = smp.tile([P, W], F32, tag=f"sq_a_{P}_{W}", name=f"sq_a_{P}_{W}")
            nc.scalar.activation(a[:], s_ap, mybir.ActivationFunctionType.Abs)
            t = smp.tile([P, W], F32, tag=f"sq_t_{P}_{W}", name=f"sq_t_{P}_{W}")
            nc.vector.tensor_tensor(t[:], s_ap, a[:], mult)
            v = smp.tile([P, W], F32, tag=f"sq_v_{P}_{W}", name=f"sq_v_{P}_{W}")
            nc.vector.tensor_tensor(v[:], t[:], r[:], mult)
            return v

        # ------------------------------------------------------------------
        def gen_phase(bg):
            a1t = a1p.tile([128, BL * N], F16, tag="a1", name="a1")
            st[("a1", bg)] = a1t
            a1v = a1t[:].rearrange("p (b g l) -> p g l b", b=BL, g=NG, l=NL)
            a2v = a2t[:].rearrange("p (G b q l) -> p q l b G",
                                   G=BG, b=BL, q=NQ, l=NL)
            xbt = xbdp.tile([128, NG * 128], F16, tag="xbd", name="xbd")
            for ch in range(3):
                c0, c1 = ch * NG // 3, (ch + 1) * NG // 3
                nc.sync.dma_start(xbt[:, c0 * 128:c1 * 128],
                                  xbd_d[bg, :, c0 * 128:c1 * 128])
            pb = None
            pa = None
            pb_q0 = 0
            pa_q0 = 0
            for Qn in range(NQ):              # 18 blocks of 4 ng
                if Qn % 2 == 0:
                    pa = psA.tile([128, 1024], F32, tag="psA", name="psA")
                    pa_q0 = Qn
                if Qn % 4 == 0:
                    pb = psB.tile([128, 512], F32, tag="psB", name="psB")
                    pb_q0 = Qn
                for j in range(4):
                    ng = 4 * Qn + j
                    w0 = wrs[:, ng * KO:ng * KO + 128]
                    w1 = wrs[:, ng * KO + 128:ng * KO + KO]
                    rhs = xbt[:, ng * 128:(ng + 1) * 128]
                    jj = (Qn - pa_q0) * 4 + j
                    nc.tensor.matmul(pa[:, jj * 128:(jj + 1) * 128],
                                     w0, rhs, start=True, stop=True)
                    q = Qn - pb_q0
                    nc.tensor.matmul(
                        pb[32 * j:32 * j + 32, q * 128:(q + 1) * 128],
                        w1, rhs, start=True, stop=True,
                        tile_position=(0, 32 * j))
                    if bg == 0:
                        nc.tensor.matmul(
                            st["m1ps"][:],
                            xsal[:, ng * BSH:(ng + 1) * BSH],
                            wrs[:, ng * KO:(ng + 1) * KO],
                            start=(ng == 0), stop=(ng == NG - 1))
                # evacuate psA -> A1[bg]; dst/src iteration order = (g, l, b)
                if Qn % 2 == 1:
                    evac_copy(a1v[:, 4 * pa_q0:4 * pa_q0 + 8, :, :],
                              pa[:].rearrange("p (g l b) -> p g l b",
                                              g=8, l=NL, b=BL))
                if Qn % 4 == 3 or Qn == NQ - 1:
                    ncnt = Qn - pb_q0 + 1
                    evac_copy(
                        a2v[:, pb_q0:pb_q0 + ncnt, :, :, bg],
                        pb[:, :ncnt * 128].rearrange(
                            "p (q l b) -> p q l b", q=ncnt, l=NL, b=BL))
                yield

        # ------------------------------------------------------------------
        def m1_finalize():
            m1s = smp.tile([32, KO], F32, tag="m1s", name="m1s")
            nc.vector.tensor_copy(m1s[:], st["m1ps"][:])
            t1 = psT.tile([128, 32], F32, tag="psT", name="psT")
            nc.tensor.transpose(t1[:], m1s[:, 0:128], id32[:])
            t2 = psT.tile([128, 32], F32, tag="psT", name="psT")
            nc.tensor.transpose(t2[:32, :], m1s[:, 128:KO], id32[:])
            s1 = smp.tile([128, 32], F32, tag="s1t1", name="s1t1")
            nc.vector.tensor_scalar_mul(s1[:], t1[:], 1.0 / N)
            v1 = squash(s1[:], 128, 32)
            st["V_t1_1"] = v1              # [128, (bg,bl)]
            s1a = smp.tile([32, 32], F32, tag="s1a2", name="s1a2")
            nc.vector.tensor_scalar_mul(s1a[:], t2[:32, :], 1.0 / N)
            v1a = squash(s1a[:], 32, 32)
            st["Va2_small_1"] = v1a        # [32(ko2), 32(bg,bl)]
            vb = psT.tile([128, 32], F32, tag="psT", name="psT")
            nc.tensor.matmul(vb[:], sel2s[:], v1a[:], start=True, stop=True)
            vbig = smp.tile([128, 32], F32, tag="va2big", name="va2big")
            nc.vector.tensor_copy(vbig[:], vb[:])
            st["Va2_big_1"] = vbig         # [128(j,ko2), 32(bg,bl)]
            yield

        # ------------------------------------------------------------------
        def route_t1(bg):
            a1t = st[("a1", bg)]
            vl = None
            for it in range(2, T + 1):
                if it == 2:
                    Vfull = st["V_t1_1"]
                    voff = 8 * bg
                else:
                    Vfull = st[("V_t1", bg, it - 1)]
                    voff = 0
                Z = smp.tile([128, BL], F32, tag="Zt1", name="Zt1")
                Y = smp.tile([128, BL], F32, tag="Yt1", name="Yt1")
                for bl in range(BL):
                    p_sl = a1t[:, bl * N:(bl + 1) * N]
                    e = ep.tile([128, N], F16, tag="e", name="e")
                    nc.scalar.activation(
                        e[:], p_sl, EXP,
                        scale=Vfull[:, voff + bl:voff + bl + 1],
                        accum_out=Z[:, bl:bl + 1])
                    scr = scrp.tile([128, N], F16, tag="scr", name="scr")
                    nc.vector.scalar_tensor_tensor(
                        out=scr[:], in0=p_sl, scalar=ones1[:], in1=e[:],
                        op0=mult, op1=mult, accum_out=Y[:, bl:bl + 1])
                    if bl % 2 == 1:
                        yield
                r = smp.tile([128, BL], F32, tag="rt1", name="rt1")
                nc.vector.reciprocal(r[:], Z[:])
                s = smp.tile([128, BL], F32, tag="st1", name="st1")
                nc.vector.tensor_tensor(s[:], Y[:], r[:], mult)
                v = squash(s[:], 128, BL)
                Vn = smp.tile([128, BL], F32, tag="Vt1n", name="Vt1n")
                nc.vector.tensor_tensor(
                    Vn[:], Vfull[:, voff:voff + BL], v[:], add)
                st[("V_t1", bg, it)] = Vn
                vl = v
                yield
            if T == 1:
                vlast = st["V_t1_1"][:, 8 * bg:8 * bg + 8]
            else:
                vlast = vl[:]
            nc.sync.dma_start(out1_d[bg], vlast)
            yield

        # ------------------------------------------------------------------
        def route_a2bg(bg):
            vl = None
            for it in range(2, T + 1):
                if it == 2:
                    Vbig = st["Va2_big_1"]
                    Vsm = st["Va2_small_1"]
                    voff = 8 * bg
                else:
                    Vbig = st[("Va2_big", bg, it - 1)]
                    Vsm = st[("Va2_small", bg, it - 1)]
                    voff = 0
                Z = smp.tile([128, BL], F32, tag="Za2", name="Za2")
                Y = smp.tile([128, BL], F32, tag="Ya2", name="Ya2")
                for bl in range(BL):
                    col = bg * BL + bl
                    off = col * NQ * NL
                    p_sl = a2t[:, off:off + NQ * NL]
                    e = ep.tile([128, N], F16, tag="e", name="e")
                    nc.scalar.activation(
                        e[:, :NQ * NL], p_sl, EXP,
                        scale=Vbig[:, voff + bl:voff + bl + 1],
                        accum_out=Z[:, bl:bl + 1])
                    scr = scrp.tile([128, N], F16, tag="scr", name="scr")
                    nc.vector.scalar_tensor_tensor(
                        out=scr[:, :NQ * NL], in0=p_sl, scalar=ones1[:],
                        in1=e[:, :NQ * NL],
                        op0=mult, op1=mult, accum_out=Y[:, bl:bl + 1])
                    if bl % 2 == 1:
                        yield
                # j-reduce Z/Y -> [32, 8]
                zr = psT.tile([128, 32], F32, tag="psT", name="psT")
                nc.tensor.matmul(zr[:32, :BL], sel1s[:], Z[:],
                                 start=True, stop=True)
                yr = psT.tile([128, 32], F32, tag="psT", name="psT")
                nc.tensor.matmul(yr[:32, :BL], sel1s[:], Y[:],
                                 start=True, stop=True)
                zs = smp.tile([32, BL], F32, tag="zs", name="zs")
                nc.vector.tensor_copy(zs[:], zr[:32, :BL])
                ys = smp.tile([32, BL], F32, tag="ys", name="ys")
                nc.vector.tensor_copy(ys[:], yr[:32, :BL])
                r = smp.tile([32, BL], F32, tag="ra2", name="ra2")
                nc.vector.reciprocal(r[:], zs[:])
                s = smp.tile([32, BL], F32, tag="sa2", name="sa2")
                nc.vector.tensor_tensor(s[:], ys[:], r[:], mult)
                v = squash(s[:], 32, BL)
                Vn = smp.tile([32, BL], F32, tag="Va2n", name="Va2n")
                nc.vector.tensor_tensor(Vn[:], Vsm[:, voff:voff + BL], v[:], add)
                st[("Va2_small", bg, it)] = Vn
                vl = v
                if it < T:
                    vb = psT.tile([128, 32], F32, tag="psT", name="psT")
                    nc.tensor.matmul(vb[:, :BL], sel2s[:], Vn[:],
                                     start=True, stop=True)
                    vbig = smp.tile([128, BL], F32, tag="va2bgn", name="va2bgn")
                    nc.vector.tensor_copy(vbig[:], vb[:, :BL])
                    st[("Va2_big", bg, it)] = vbig
                yield
            if T == 1:
                vlast = st["Va2_small_1"][:, 8 * bg:8 * bg + 8]
            else:
                vlast = vl[:]
            nc.sync.dma_start(out2_d[:, 8 * bg:8 * bg + 8], vlast)
            yield

        # ---- emission schedule (pipelined: route(bg-1) || gen(bg)) ----
        import os
        phase = os.environ.get("KCAP_PHASE", "full")
        st["m1ps"] = psM.tile([32, KO], F32, tag="m1ps", name="m1ps")
        if phase == "gen":
            _interleave(gen_phase(0))
            for bg in range(1, BG):
                _interleave(gen_phase(bg))
            # dump a slice of A1/A2 so nothing is dead
            dbg = smp.tile([128, 32], F32, tag="dbg", name="dbg")
            nc.vector.tensor_copy(dbg[:], st[("a1", 0)][:, 0:32])
            nc.sync.dma_start(out2_d, dbg[:32, :])
            nc.sync.dma_start(out1_d[0], dbg[:, :8])
        elif phase == "m1":
            _interleave(gen_phase(0))
            _interleave(m1_finalize())
            for bg in range(1, BG):
                _interleave(gen_phase(bg))
            nc.sync.dma_start(out1_d[0], st["V_t1_1"][:, :8])
            nc.sync.dma_start(out2_d, st["Va2_small_1"][:])
        else:
            _interleave(gen_phase(0))
            _interleave(m1_finalize())
            for bg in range(1, BG):
                _interleave(gen_phase(bg), route_t1(bg - 1),
                            route_a2bg(bg - 1))
            _interleave(route_t1(BG - 1), route_a2bg(BG - 1))

    nc.compile()
    return nc


def _get_program(T):
    if T not in _cache:
        _cache[T] = _build_program(T)
    return _cache[T]


# ----------------------------------------------------------------------------
# host-side output assembly
# ----------------------------------------------------------------------------
def _assemble(results):
    v = np.zeros((K, B, 1, 1, O), dtype=np.float32)
    for c, res in enumerate(results):
        o1 = res["out1"]          # [BG, 128=(16k+o), BL]
        o2 = res["out2"]          # [32=(16(k-8)+o), 32=(bg,bl)]
        b0 = c * BSH
        # o1[bg, 16k+o, bl] -> v[k, b0+8bg+bl, 0, 0, o]
        t = o1.reshape(BG, 8, O, BL).transpose(1, 0, 3, 2)  # [k, bg, bl, o]
        v[:8, b0:b0 + BSH, 0, 0, :] = t.reshape(8, BSH, O)
        # o2[16kk+o, 8bg+bl] -> v[8+kk, b0+8bg+bl, 0, 0, o]
        t2 = o2.reshape(2, O, BSH).transpose(0, 2, 1)       # [kk, b, o]
        v[8:, b0:b0 + BSH, 0, 0, :] = t2
    return v


def run(x, routing_weights, num_iterations, trace=False):
    T = int(num_iterations)
    x = np.asarray(x, dtype=np.float32)
    w = np.asarray(routing_weights, dtype=np.float32)
    nc = _get_program(T)
    in_maps = _prep_core_inputs(x, w)
    kw = {}
    if trace:
        kw = dict(trace=True, trace_cores=list(range(NCORES)))
    res = run_bass_kernel_spmd(nc, in_maps, core_ids=list(range(NCORES)), **kw)
    return _assemble(res.results), res


def kernel(x, routing_weights, num_iterations):
    out, _ = run(x, routing_weights, num_iterations)
    return out
